# revision 1
# baseline (speedup 1.0000x reference)
"""Trainium2 Bass kernel for nn_AdvancedIQCNN.

Pipeline (per sample):
  h  = relu(bn(x @ W1.T + b1)) ; h = relu(bn(h @ W2.T + b2))   (BN over full batch)
  xq = quantum(h)                                              (13-qubit circuits)
  out = relu(xq@W3.T+b3) -> relu(@W4.T+b4) -> @W5.T+b5

The quantum layer is evaluated in closed form. Propagating the measured
observable P(qubit0=1) backward through the shallow circuits (Heisenberg
picture, CX-chain conjugation of the Pauli string) collapses the 2^13
statevector simulation to 6 terms built from sin/cos of h[:, 0:3]:

  xq = k0 + k1*cos(h0) + k2*sin(h0)sin(h1) + k3*sin(h0)sin(h2)
          + k4*cos(h0)sin(h1)sin(h2) + k5*cos(h1)

with k* precomputed from the (replicated, tiny) theta angles.

Sharding: pure data parallel over 8 cores. Each core receives the full x
batch-rotated so its own 512-sample shard sits in packed slot 0; every core
redundantly computes the (tiny) front MLP over the full batch to get exact
BatchNorm statistics without collectives, then runs the quantum closed form
and back MLP on its shard only.

Performance structure:
  - 4 batch chunks are packed along the partition dim with block-diagonal
    weights (K=4*13=52/4*26=104 <= 128), so one matmul and one evac/square/
    relu instruction process 4 chunks at once; the front MLP over the full
    4096-batch is 2 matmuls per layer instead of 8. Exact fp32 throughout
    (the block-diagonal zeros contribute exact 0.0 to the fp32 dot products).
  - BN statistics come from accum_out side channels of the PSUM-evacuation
    (sum) and an ACT Square pass (sum of squares); partition-packed partials
    are folded AND replicated back to the packed layout in one matmul with a
    block-of-identities matrix, so all stat math runs in packed form.
  - rstd uses a DVE Newton rsqrt (bit-hack seed + 3 iterations), keeping
    every ACT func inside the single trig_and_small table (one table load,
    triggered early by a dummy Sin, overlapped with the input DMAs).
  - quantum term rows are built with small selection matmuls on the PE.
  - one packed weights/consts DMA + one x DMA (2 chunks); a dummy matmul
    warms the PE p-state during the input DMAs.
"""

import sys

if "/opt/trn_rl_repo" not in sys.path:
    sys.path.insert(0, "/opt/trn_rl_repo")

from contextlib import ExitStack

import numpy as np

B = 4096
NF = 13
NCORES = 8
SH = B // NCORES  # 512 samples per core
CH = 512
PK = 4            # chunks packed along partitions
NJ = B // (CH * PK)  # 2 column blocks
K1 = PK * NF      # 52
K2 = PK * 26      # 104

# wpack column layout ([K2=104] partitions x WCOLS fp32)
_C = {}
_o = 0


def _col(name, n):
    global _o
    _C[name] = (_o, _o + n)
    _o += n


_col("W1BD", K2)   # [52, 104] block-diag of W1.T [13,26] x4
_col("W2BD", K1)   # [104, 52] block-diag of W2.T [26,13] x4
_col("W3T", 32)    # [1, 32]
_col("W4T", 16)    # [32, 16]
_col("W5T", 2)     # [16, 2]
_col("KC", 1)      # [6, 1]
_col("B1R", 1)     # [104, 1] b1 tiled x4
_col("B2R", 1)     # [52, 1]  b2 tiled x4
_col("B3", 1)      # [32, 1]
_col("B4", 1)      # [16, 1]
_col("B5", 1)      # [2, 1]
_col("G1R", 1)     # [104, 1] g1 tiled x4
_col("BE1R", 1)
_col("G2R", 1)     # [52, 1]
_col("BE2R", 1)
_col("SR1", K2)    # [104, 104] block-of-identities: fold+replicate in one
_col("SR2", K1)    # [52, 52]
_col("D36", 6)     # [3, 6] duplication selection
_col("S1", 6)      # [6, 6] M1 selection
_col("S2", 6)
_col("S3", 6)
_col("SINB", 1)    # [6, 1] sin biases [pi,pi,pi,pi/2,pi/2,pi/2]
_col("M1S", 1)     # [6, 1] evac scale/bias columns
_col("M1B", 1)
_col("M2S", 1)
_col("M2B", 1)
_col("M3S", 1)
_col("M3B", 1)
_col("EPS", 1)     # 1e-5
WCOLS = _o


def _build_nc(reps=1, loop_n=1):
    import concourse.bass as bass
    import concourse.mybir as mybir
    import concourse.tile as tile
    from concourse import bacc

    dt = mybir.dt.float32
    AF = mybir.ActivationFunctionType
    AL = mybir.AluOpType
    ts = bass.ts

    nc = bacc.Bacc("TRN2", target_bir_lowering=False, debug=False)

    xS = nc.dram_tensor("xS", [K1, NJ * CH], dt, kind="ExternalInput").ap()
    wp = nc.dram_tensor("wp", [K2, WCOLS], dt, kind="ExternalInput").ap()
    outT = nc.dram_tensor("outT", [2, SH], dt, kind="ExternalOutput").ap()

    with tile.TileContext(nc) as tc, ExitStack() as ctx:
        pool = ctx.enter_context(tc.tile_pool(name="sb", bufs=1))
        sqp = ctx.enter_context(tc.tile_pool(name="sq", bufs=2))
        psum = ctx.enter_context(tc.tile_pool(name="ps", bufs=7, space="PSUM"))

        for i, val in enumerate((0.0,)):
            t = pool.tile([128, 1], dt, tag=f"const{i}")
            nc.vector.memset(t[:], val)
            nc.const_aps.aps[(dt, val)] = t[:]

        magic = pool.tile([128, 1], dt, tag="magic")
        nc.vector.memset(magic[:].bitcast(mybir.dt.int32), 0x5F3759DF)

        # dummy Sin on a const tile: triggers the single trig_and_small ACT
        # table load early, overlapped with the input DMAs (Square/Relu/
        # Identity/Copy/Sin all live in that one table; Sqrt is avoided below)
        sdum = pool.tile([1, 1], dt, tag="sdum")
        nc.scalar.activation(sdum[:], t[0:1, :], AF.Sin)

        # PE p-state warm-up: one long dummy matmul keeps the PE busy during
        # the input DMAs so the real matmuls run at full clock.
        wrm = pool.tile([1, CH + 1], dt, tag="wrm")
        nc.gpsimd.memset(wrm[:], 0.0)
        pwm = psum.tile([1, CH], dt, tag="warm", bufs=1)
        nc.tensor.matmul(pwm[:], wrm[0:1, 0:1], wrm[0:1, 1 : CH + 1])

        # DMA issue order matters: the SP sequencer issues ~650ns apart, so
        # the L1-gating transfer (x block 0) goes first.
        w = pool.tile([K2, WCOLS], dt, tag="wp")
        xsb = pool.tile([K1, NJ * CH], dt, tag="xsb")
        nc.sync.dma_start(out=xsb[:, ts(0, CH)], in_=xS[:, ts(0, CH)])
        nc.sync.dma_start(out=w[:], in_=wp[:])
        nc.sync.dma_start(out=xsb[:, ts(1, CH)], in_=xS[:, ts(1, CH)])

        def W(name, p):
            lo, hi = _C[name]
            return w[0:p, lo:hi]

        def mm(out_ap, lhsT, rhs, **kw):
            nc.tensor.matmul(out_ap, lhsT, rhs, **kw)

        def packed_bn_layer(in_sb, kin, wname, kout, fout, brname, sumname,
                            gname, bename, lname):
            """Packed z = blockdiag(wT).T@in + b; BN stats via accum_out.
            Returns (z tile [kout, NJ*CH], scale [fout,1], shift [fout,1])."""
            z = pool.tile([kout, NJ * CH], dt, tag=f"z{lname}")
            parts = pool.tile([kout, 2 * NJ], dt, tag=f"parts{lname}")
            bcol = W(brname, kout)
            for j in range(NJ):
                pm = psum.tile([kout, CH], dt, tag="mm")
                mm(pm[:], W(wname, kin), in_sb[:, ts(j, CH)])
                # evac + bias; accum -> per-packed-row sum partial (col j)
                nc.vector.tensor_scalar(
                    z[:, ts(j, CH)], pm[:], bcol, None, op0=AL.add, op1=AL.add,
                    accum_out=parts[:, j : j + 1],
                )
                # (z)^2 straight from PSUM; accum -> sumsq partial (col NJ+j)
                sq = sqp.tile([kout, CH], dt, tag="sqscr")
                nc.scalar.activation(
                    sq[:], pm[:], AF.Square, bias=bcol,
                    accum_out=parts[:, NJ + j : NJ + j + 1],
                )
            # fold the PK partition groups AND replicate back in one matmul:
            # SR = (stacked I) @ (repeated I) has I_fout in every block, so
            # pf[r] = group-sum for r's feature, already in packed layout.
            fout = kout
            pf = psum.tile([fout, 2 * NJ], dt, tag="mm")
            mm(pf[:], W(sumname, kout), parts[:])
            st = pool.tile([fout, 2 * NJ], dt, tag=f"st{lname}")
            nc.vector.tensor_scalar_add(st[:], pf[:], 0.0)
            # reduce the NJ column blocks: view [fout, 2, NJ] -> [fout, 2]
            tot = pool.tile([fout, 2], dt, tag=f"tot{lname}")
            nc.vector.reduce_sum(
                tot[:], st[:].rearrange("p (k j) -> p k j", k=2),
                axis=mybir.AxisListType.X,
            )
            mean = pool.tile([fout, 1], dt, tag=f"mean{lname}")
            nc.vector.tensor_scalar_mul(mean[:], tot[:, 0:1], 1.0 / B)
            m2 = pool.tile([fout, 1], dt, tag=f"m2{lname}")
            nc.vector.tensor_mul(m2[:], mean[:], mean[:])
            var = pool.tile([fout, 1], dt, tag=f"var{lname}")
            nc.vector.scalar_tensor_tensor(
                var[:], tot[:, 1:2], 1.0 / B, m2[:], op0=AL.mult, op1=AL.subtract
            )
            # rstd = (var+eps)^-1/2 via bit-hack seed + 3 Newton iterations
            # on the DVE (exact to ~1e-11 rel) -- avoids ACT Sqrt, which lives
            # in a different activation table than Sin and would force two
            # extra ~1.3us table reloads (one on the tail critical path).
            xve = pool.tile([fout, 1], dt, tag=f"xve{lname}")
            nc.vector.tensor_scalar_add(xve[:], var[:], W("EPS", fout))
            i32 = mybir.dt.int32
            yi = pool.tile([fout, 1], dt, tag=f"yi{lname}")
            nc.vector.tensor_scalar(
                yi[:].bitcast(i32), xve[:].bitcast(i32), 1, None,
                op0=AL.logical_shift_right,
            )
            nc.vector.scalar_tensor_tensor(
                yi[:].bitcast(i32), magic[0:fout, :].bitcast(i32), 1,
                yi[:].bitcast(i32), op0=AL.mult, op1=AL.subtract,
            )
            rstd = yi
            ya = pool.tile([fout, 1], dt, tag=f"ya{lname}")
            yb = pool.tile([fout, 1], dt, tag=f"yb{lname}")
            for _ in range(3):
                nc.vector.tensor_mul(ya[:], rstd[:], rstd[:])
                nc.vector.tensor_mul(yb[:], xve[:], ya[:])
                nc.vector.tensor_scalar(
                    yb[:], yb[:], -0.5, 1.5, op0=AL.mult, op1=AL.add
                )
                nc.vector.tensor_mul(rstd[:], rstd[:], yb[:])
            scale = pool.tile([fout, 1], dt, tag=f"scale{lname}")
            nc.vector.tensor_mul(scale[:], rstd[:], W(gname, fout))
            shift = pool.tile([fout, 1], dt, tag=f"shift{lname}")
            nc.vector.tensor_mul(shift[:], mean[:], scale[:])
            nc.vector.tensor_sub(shift[:], W(bename, fout), shift[:])
            return z, scale, shift

        def body():
            z1, sc1, sh1 = packed_bn_layer(
                xsb, K1, "W1BD", K2, 26, "B1R", "SR1", "G1R", "BE1R", "1"
            )
            h1 = pool.tile([K2, NJ * CH], dt, tag="h1")
            for j in range(NJ):
                nc.scalar.activation(
                    h1[:, ts(j, CH)], z1[:, ts(j, CH)], AF.Relu,
                    bias=sh1[:], scale=sc1[:],
                )

            return packed_bn_layer(
                h1, K2, "W2BD", K1, NF, "B2R", "SR2", "G2R", "BE2R", "2"
            )

        def tail(z2, sc2, sh2):
            # ---- quantum closed form on features 0..2 of the local shard ----
            # local shard = packed slot 0 = partitions 0:13 of column block 0
            hq = pool.tile([3, SH], dt, tag="hq")
            nc.scalar.activation(
                hq[:], z2[0:3, 0:SH], AF.Relu, bias=sh2[0:3, :], scale=sc2[0:3, :]
            )
            # duplicate to 6 rows; scc = sin(SINB - hq6) = [s0,s1,s2,c0,c1,c2]
            p6 = psum.tile([6, SH], dt, tag="mm")
            mm(p6[:], W("D36", 3), hq[:])
            scc = pool.tile([6, SH], dt, tag="scc")
            nc.scalar.activation(scc[:], p6[:], AF.Sin, bias=W("SINB", 6), scale=-1.0)

            # M1=[1,c0,c1,s0,s0,c0], M2=[1,1,1,s1,s2,s2], M3=[1,1,1,1,1,s1]
            Ms = []
            for sname, scl, bia in (("S1", "M1S", "M1B"), ("S2", "M2S", "M2B"),
                                    ("S3", "M3S", "M3B")):
                pm = psum.tile([6, SH], dt, tag="mm")
                mm(pm[:], W(sname, 6), scc[:])
                m = pool.tile([6, SH], dt, tag=f"m{sname}")
                # ACT, not DVE: the DVE queue is busy with the L2 Newton
                # stats chain right when these become ready
                nc.scalar.activation(
                    m[:], pm[:], AF.Identity, bias=W(bia, 6), scale=W(scl, 6)
                )
                Ms.append(m)
            T = pool.tile([6, SH], dt, tag="T")
            nc.vector.tensor_mul(T[:], Ms[0][:], Ms[1][:])
            nc.vector.tensor_mul(T[:], T[:], Ms[2][:])

            xqp = psum.tile([1, SH], dt, tag="mm")
            mm(xqp[:], W("KC", 6), T[:])
            xq = pool.tile([1, SH], dt, tag="xq")
            nc.vector.tensor_scalar_add(xq[:], xqp[:], 0.0)

            # ---- back MLP ----
            z3 = psum.tile([32, SH], dt, tag="mm")
            mm(z3[:], W("W3T", 1), xq[:])
            h3 = pool.tile([32, SH], dt, tag="h3")
            nc.scalar.activation(h3[:], z3[:], AF.Relu, bias=W("B3", 32))
            z4 = psum.tile([16, SH], dt, tag="mm")
            mm(z4[:], W("W4T", 32), h3[:])
            h4 = pool.tile([16, SH], dt, tag="h4")
            nc.scalar.activation(h4[:], z4[:], AF.Relu, bias=W("B4", 16))
            z5 = psum.tile([2, SH], dt, tag="mm")
            mm(z5[:], W("W5T", 16), h4[:])
            o = pool.tile([2, SH], dt, tag="o")
            nc.scalar.activation(o[:], z5[:], AF.Identity, bias=W("B5", 2))
            nc.sync.dma_start(out=outT[:], in_=o[:])

        if loop_n > 1:
            with tc.For_i(0, loop_n, 1):
                tail(*body())
        else:
            for _rep in range(reps):
                tail(*body())

    nc.compile()
    return nc


def _wpack(inputs):
    f32 = np.float32
    a, b, t = (
        np.asarray(inputs["th1a"], f32),
        np.asarray(inputs["th1b"], f32),
        np.asarray(inputs["th2a"], f32),
    )
    ca0, sa0 = np.cos(a[0]), np.sin(a[0])
    ca1, sa1 = np.cos(a[1]), np.sin(a[1])
    cb0, sb0 = np.cos(b[0]), np.sin(b[0])
    ct0, st0 = np.cos(t[0]), np.sin(t[0])
    # xq = 0.5 - (E1+E2)/4, T rows = [1, c0, c1, s0s1, s0s2, c0s1s2]
    kcv = np.array(
        [
            0.5,
            -(cb0 * ca0 + ct0) / 4.0,
            (sb0 * sa0 * sa1) / 4.0,
            (cb0 * sa0 + st0) / 4.0,
            (sb0 * ca0 * ca1) / 4.0,
            (sb0 * sa0 * ca1) / 4.0,
        ],
        f32,
    )

    wpk = np.zeros((K2, WCOLS), f32)

    def put(name, arr):
        lo, hi = _C[name]
        arr = np.asarray(arr, f32)
        if arr.ndim == 1:
            arr = arr[:, None]
        wpk[: arr.shape[0], lo:hi] = arr

    w1t = np.asarray(inputs["W1"], f32).T  # [13, 26]
    w2t = np.asarray(inputs["W2"], f32).T  # [26, 13]
    w1bd = np.zeros((K1, K2), f32)
    w2bd = np.zeros((K2, K1), f32)
    sr1 = np.tile(np.eye(26, dtype=f32), (PK, PK))
    sr2 = np.tile(np.eye(NF, dtype=f32), (PK, PK))
    for c in range(PK):
        w1bd[c * NF : (c + 1) * NF, c * 26 : (c + 1) * 26] = w1t
        w2bd[c * 26 : (c + 1) * 26, c * NF : (c + 1) * NF] = w2t
    put("W1BD", w1bd)
    put("W2BD", w2bd)
    put("SR1", sr1)
    put("SR2", sr2)
    put("W3T", np.asarray(inputs["W3"], f32).T)
    put("W4T", np.asarray(inputs["W4"], f32).T)
    put("W5T", np.asarray(inputs["W5"], f32).T)
    put("KC", kcv)
    put("B1R", np.tile(np.asarray(inputs["b1"], f32), PK))
    put("B2R", np.tile(np.asarray(inputs["b2"], f32), PK))
    put("B3", inputs["b3"]); put("B4", inputs["b4"]); put("B5", inputs["b5"])
    put("G1R", np.tile(np.asarray(inputs["g1"], f32), PK))
    put("BE1R", np.tile(np.asarray(inputs["beta1"], f32), PK))
    put("G2R", np.tile(np.asarray(inputs["g2"], f32), PK))
    put("BE2R", np.tile(np.asarray(inputs["beta2"], f32), PK))
    d36 = np.zeros((3, 6), f32)
    for m in range(6):
        d36[m % 3, m] = 1.0
    put("D36", d36)
    # scc rows: [s0, s1, s2, c0, c1, c2]
    s1m = np.zeros((6, 6), f32)
    for m, k in ((1, 3), (2, 4), (3, 0), (4, 0), (5, 3)):
        s1m[k, m] = 1.0
    put("S1", s1m)
    s2m = np.zeros((6, 6), f32)
    for m, k in ((3, 1), (4, 2), (5, 2)):
        s2m[k, m] = 1.0
    put("S2", s2m)
    s3m = np.zeros((6, 6), f32)
    s3m[1, 5] = 1.0
    put("S3", s3m)
    put("SINB", np.array([np.pi] * 3 + [np.pi / 2] * 3, f32))
    put("M1S", np.array([0, 1, 1, 1, 1, 1], f32))
    put("M1B", np.array([1, 0, 0, 0, 0, 0], f32))
    put("M2S", np.array([0, 0, 0, 1, 1, 1], f32))
    put("M2B", np.array([1, 1, 1, 0, 0, 0], f32))
    put("M3S", np.array([0, 0, 0, 0, 0, 1], f32))
    put("M3B", np.array([1, 1, 1, 1, 1, 0], f32))
    put("EPS", np.full(K2, 1e-5, f32))
    return wpk


def _in_maps(inputs):
    x = np.ascontiguousarray(np.asarray(inputs["x"], np.float32))
    wpk = _wpack(inputs)
    maps = []
    for c in range(NCORES):
        xr = np.roll(x, -c * SH, axis=0)
        # packed layout: xS[13*cc + f, 512*j + n] = xr[512*(PK*j + cc) + n, f]
        xs = xr.reshape(NJ, PK, CH, NF).transpose(1, 3, 0, 2).reshape(K1, NJ * CH)
        maps.append({"xS": np.ascontiguousarray(xs), "wp": wpk})
    return maps


def run_spmd(inputs, **kw):
    from concourse import bass_utils

    nc = _build_nc()
    res = bass_utils.run_bass_kernel_spmd(nc, _in_maps(inputs), list(range(NCORES)), **kw)
    out = np.concatenate([res.results[c]["outT"].T for c in range(NCORES)], axis=0)
    return out.astype(np.float32), res


def kernel(**inputs):
    return run_spmd(inputs)[0]


if __name__ == "__main__":
    print("built nc ok:", _build_nc() is not None)



# revision 21
# speedup vs baseline: 4.4717x; 4.4717x over previous
"""Trainium2 Bass kernel for nn_AdvancedIQCNN (v2).

Pipeline (per sample):
  h  = relu(bn(x @ W1.T + b1)) ; h = relu(bn(h @ W2.T + b2))   (BN over full batch)
  xq = quantum(h)                                              (13-qubit circuits)
  out = relu(xq@W3.T+b3) -> relu(@W4.T+b4) -> @W5.T+b5

The quantum layer reduces (Heisenberg picture) to a 6-term closed form in
sin/cos of h[:, 0:3]; v2 pushes it further with product-to-sum identities:

  xq = k0 + sum_j alpha_j * cos(u_j),   u = L @ h[0:3]   (10 cosine terms)

so the whole tail is: relu -> matmul(L) -> Sin-act -> matmul(W3*alpha,
k0 folded into b3) -> relu -> matmul(W4) -> relu -> matmul(W5) -> +b5.

Key v2 optimizations over v1 (49.5us -> target <15us):
  - float32r matmuls: 1 cycle/row at >=256 output columns (4x over fp32).
  - BN stats without evacuating z: variance is translation-invariant so the
    Square pass needs no bias; means come from algebra (mean z1 from column
    sums of x through -W1@SR1/B, mean z2 free from the relu pass accum_out).
    BN makes b1/b2 irrelevant entirely (batch-stat mode subtracts them).
  - beta=0 / gamma>0 fast path (chosen host-side from actual input values,
    exact): relu commutes with the positive BN scale, so relu only waits on
    the mean (not the Newton rsqrt); the scale folds into the next matmul's
    weights (W2 row-scale on chip, L-matrix row-scale for the quantum tail).
  - tail packed 4x128 along partitions in fp16 (53ns matmuls, 128-col ACTs),
    one PSUM bank holds args/h3/h4/out in disjoint column ranges.
  - engine balance: ACT does sq/relu block 0 + tail, DVE does x-reduce,
    relu block 1 (one 2-op tensor_scalar), stats Newton; Pool (gpsimd) does
    block-1 squares and tiny PSUM evacuations.
  - 2-body (A/B) unroll inside the hardware loop so iteration k+1's front
    overlaps iteration k's tail (separate tail buffers, shared z PSUM).

Sharding: pure data parallel over 8 cores as v1: every core redundantly
computes the front MLP over the full 4096 batch (exact BN stats without
collectives), runs the tail on its own 512-sample shard.
"""

import sys

if "/opt/trn_rl_repo" not in sys.path:
    sys.path.insert(0, "/opt/trn_rl_repo")

from contextlib import ExitStack

import numpy as np

B = 4096
NF = 13
NCORES = 8
SH = B // NCORES  # 512 samples per core
CH = 512
PK = 4            # batch chunks packed along partitions (front)
NJ = B // (CH * PK)  # 2 column blocks of 512
K1 = PK * NF      # 52
K2 = PK * 26      # 104
TC = 4            # tail chunks of 128 cols packed along partitions
TN = SH // TC     # 128

# f32 weights tensor wp: [104 partitions x WCOLS]
_C = {}
_o = 0


def _col(name, n):
    global _o
    _C[name] = (_o, _o + n)
    _o += n


_col("W1BD", K2)    # [52, 104] block-diag W1.T x4 (f32r at matmul)
_col("W2BD", K1)    # [104, 52] block-diag W2.T x4 (row-scaled on chip)
_col("SR1", K2)     # [104, 104] block-of-identities / B (fold+replicate)
_col("SR2", K1)     # [52, 52] / B
_col("SR1N", K2)    # -SR1: mean folds produce t'' = -E[z] directly
_col("SR2N", K1)
_col("LMAT", 10)    # [3, 10] trig linear map L.T (f32; scaled by s2 on chip)
_col("G1R", 1)      # [104,1] gamma1 tiled x4 (general path / g!=1)
_col("G2R", 1)      # [52,1]
_col("BE1R", 1)     # [104,1] beta1 tiled (general path)
_col("BE2R", 1)
WCOLS = _o

# fp16 tail weights tensor wb: [128 partitions x BCOLS]
_CB = {}
_ob = 0


def _colb(name, n):
    global _ob
    _CB[name] = (_ob, _ob + n)
    _ob += n


_colb("W3KBD", TN)   # [40, 128] block-diag (W3 x alpha).T x4
_colb("W4BD", 16 * TC)  # [128, 64] block-diag W4.T x4
_colb("W5BD", 2 * TC)   # [64, 8] block-diag W5.T x4
_colb("LMATH", 10)   # [3, 10] L.T in fp16 (general path lhsT)
BCOLS = _ob

# f32 tail consts wc: [128 partitions x 3]
# col0: b3' = b3 + W3[:,0]*k0 tiled x4 [128]; col1: b4 x4 [64]; col2: b5 x4 [8]


def _build_nc(reps=1, loop_n=1, fast_path=True):
    import concourse.bass as bass
    import concourse.mybir as mybir
    import concourse.tile as tile
    from concourse import bacc

    dt = mybir.dt.float32
    f32r = mybir.dt.float32r
    f16 = mybir.dt.float16
    bf16 = mybir.dt.bfloat16
    i32 = mybir.dt.int32
    AF = mybir.ActivationFunctionType
    AL = mybir.AluOpType
    ts = bass.ts

    nc = bacc.Bacc("TRN2", target_bir_lowering=False, debug=False)

    xS = nc.dram_tensor("xS", [K1, NJ * CH], f32r, kind="ExternalInput").ap()
    wr = nc.dram_tensor("wr", [K1, K2], f32r, kind="ExternalInput").ap()
    wp = nc.dram_tensor("wp", [K2, WCOLS], dt, kind="ExternalInput").ap()
    wb = nc.dram_tensor("wb", [128, BCOLS], f16, kind="ExternalInput").ap()
    wc = nc.dram_tensor("wc", [128, 3], dt, kind="ExternalInput").ap()
    outT = nc.dram_tensor("outT", [2 * TC, TN], dt, kind="ExternalOutput").ap()

    with tile.TileContext(nc) as tc, ExitStack() as ctx:
        pool = ctx.enter_context(tc.tile_pool(name="sb", bufs=1))
        psum = ctx.enter_context(tc.tile_pool(name="ps", bufs=7, space="PSUM"))
        wrmp = ctx.enter_context(tc.tile_pool(name="wr", bufs=1, space="PSUM"))

        for i, val in enumerate((0.0, float(np.pi / 2))):
            t = pool.tile([128, 1], dt, tag=f"const{i}")
            nc.vector.memset(t[:], val)
            nc.const_aps.aps[(dt, val)] = t[:]

        magic = pool.tile([128, 1], dt, tag="magic")
        nc.vector.memset(magic[:].bitcast(i32), 0x5F3759DF)

        # trig_and_small table preload (Sin/Relu/Square/Identity/Copy all
        # live there), overlapped with the input DMAs
        sdum = pool.tile([1, 1], dt, tag="sdum")
        nc.scalar.activation(sdum[:], t[0:1, :], AF.Sin)

        # PE p-state warm-up
        wrm = pool.tile([1, CH + 1], dt, tag="wrm")
        nc.gpsimd.memset(wrm[:], 0.0)
        pwm = wrmp.tile([1, CH], dt, tag="warm", bufs=1)
        nc.tensor.matmul(pwm[:], wrm[0:1, 0:1], wrm[0:1, 1 : CH + 1])

        w = pool.tile([K2, WCOLS], dt, tag="wp")
        w1r = pool.tile([K1, K2], f32r, tag="w1r")
        wbt = pool.tile([128, BCOLS], f16, tag="wb")
        wct = pool.tile([128, 3], dt, tag="wc")
        xsb = pool.tile([K1, NJ * CH], f32r, tag="xsb")
        nc.sync.dma_start(out=xsb[:, ts(0, CH)], in_=xS[:, ts(0, CH)])
        nc.sync.dma_start(out=w[:], in_=wp[:])
        nc.sync.dma_start(out=xsb[:, ts(1, CH)], in_=xS[:, ts(1, CH)])
        nc.sync.dma_start(out=w1r[:], in_=wr[:])
        nc.sync.dma_start(out=wbt[:], in_=wb[:])
        nc.sync.dma_start(out=wct[:], in_=wc[:])

        def W(name, p):
            lo, hi = _C[name]
            return w[0:p, lo:hi]

        def WB(name, p):
            lo, hi = _CB[name]
            return wbt[0:p, lo:hi]

        # shared PSUM tiles (A/B bodies share; Tile tracks WAR hazards);
        # bufs=1 each: exactly 7 banks total
        z1P = [psum.tile([K2, CH], dt, name=f"z1{j}", tag=f"z1{j}", bufs=1) for j in range(NJ)]
        z2P = [psum.tile([K1, CH], dt, name=f"z2{j}", tag=f"z2{j}", bufs=1) for j in range(NJ)]
        statP = psum.tile([K2, 16], dt, name="statP", tag="statP", bufs=1)
        tailP = [
            psum.tile([128, 512], dt, name=f"tailP{s}", tag=f"tailP{s}", bufs=1) for s in range(2)
        ]

        def mm(out_ap, lhsT, rhs, **kw):
            nc.tensor.matmul(out_ap, lhsT, rhs, **kw)

        def newton2(sfx, name, xve, fout, eng):
            """rsqrt(xve) via bit-hack seed + 2 Newton iterations on eng."""
            yi = pool.tile([fout, 1], dt, tag=f"yi{name}{sfx}")
            y2 = pool.tile([fout, 1], dt, tag=f"y2{name}{sfx}")
            tt = pool.tile([fout, 1], dt, tag=f"tt{name}{sfx}")
            eng.tensor_scalar(
                yi[:].bitcast(i32), xve[:].bitcast(i32), 1, None,
                op0=AL.logical_shift_right,
            )
            eng.scalar_tensor_tensor(
                yi[:].bitcast(i32), magic[0:fout, :].bitcast(i32), 1,
                yi[:].bitcast(i32), op0=AL.mult, op1=AL.subtract,
            )
            for _ in range(2):
                eng.tensor_mul(y2[:], yi[:], yi[:])
                eng.scalar_tensor_tensor(
                    tt[:], xve[:], -0.5, y2[:], op0=AL.mult, op1=AL.mult
                )
                eng.scalar_tensor_tensor(
                    yi[:], tt[:], 1.5, yi[:], op0=AL.add, op1=AL.mult
                )
            return yi

        def newton1(sfx, name, xve, fout, eng):
            """rsqrt(xve): bit-hack seed + 1 Newton iteration (rel ~2e-3)."""
            yi = pool.tile([fout, 1], dt, tag=f"yi{name}{sfx}")
            y2 = pool.tile([fout, 1], dt, tag=f"y2{name}{sfx}")
            tt = pool.tile([fout, 1], dt, tag=f"tt{name}{sfx}")
            eng.tensor_scalar(
                yi[:].bitcast(i32), xve[:].bitcast(i32), 1, None,
                op0=AL.logical_shift_right,
            )
            eng.scalar_tensor_tensor(
                yi[:].bitcast(i32), magic[0:fout, :].bitcast(i32), 1,
                yi[:].bitcast(i32), op0=AL.mult, op1=AL.subtract,
            )
            eng.tensor_mul(y2[:], yi[:], yi[:])
            eng.scalar_tensor_tensor(
                tt[:], xve[:], -0.5, y2[:], op0=AL.mult, op1=AL.mult
            )
            eng.scalar_tensor_tensor(
                yi[:], tt[:], 1.5, yi[:], op0=AL.add, op1=AL.mult
            )
            return yi

        def front(sfx, bix):
            """BN front for one iteration; returns state for tail().

            ACT : evac1b+acc, relu1a, evac2a+acc  (+552 aux)
            DVE : evac1a+acc, sq1a+acc, relu1b, evac2b+acc, sq2a+acc,
                  full stats chains (copies, var, Newton-1, W2p)
            Pool: sq1b+acc, sq2b+acc, partial adds
            PE  : mm1 x2, mm2 x2 (f32r), 4 fold matmuls
            The z evacuations' accum_out give sum(z) free (variance is
            translation-invariant; b1/b2 cancel in batch-stat BN).
            """
            pt = lambda shape, d, tag: pool.tile(shape, d, name=f"{tag}{sfx}", tag=f"{tag}{sfx}")
            c0 = 4 * bix  # statP column base for this body

            for j in range(NJ):
                mm(z1P[j][:], w1r[:], xsb[:, ts(j, CH)])

            z1sb = pt([K2, NJ * CH], dt, "z1sb")
            sums1 = pt([K2, 4], dt, "sums1")  # sz_a, sz_b, ss_a, ss_b
            nc.vector.tensor_scalar(
                z1sb[:, ts(0, CH)], z1P[0][:], 0.0, 0.0, op0=AL.add,
                op1=AL.add, accum_out=sums1[:, 0:1],
            )
            nc.scalar.activation(
                z1sb[:, ts(1, CH)], z1P[1][:], AF.Copy,
                accum_out=sums1[:, 1:2],
            )
            s1tot = pt([K2, 1], dt, "s1tot")
            nc.gpsimd.tensor_add(s1tot[:], sums1[:, 0:1], sums1[:, 1:2])
            mm(statP[:, c0 : c0 + 1], W("SR1N", K2), s1tot[:])

            sqa = pt([K2, CH], bf16, "sq1a")
            nc.vector.scalar_tensor_tensor(
                sqa[:], z1sb[:, ts(0, CH)], 0.0, z1sb[:, ts(0, CH)],
                op0=AL.add, op1=AL.mult, accum_out=sums1[:, 2:3],
            )
            sqb = pt([K2, CH], bf16, "sq1b")
            nc.scalar.activation(
                sqb[:], z1P[1][:], AF.Square, accum_out=sums1[:, 3:4]
            )
            ps1 = pt([K2, 1], dt, "ps1")
            nc.gpsimd.tensor_add(ps1[:], sums1[:, 2:3], sums1[:, 3:4])
            mm(statP[:, c0 + 1 : c0 + 2], W("SR1", K2), ps1[:])

            # stats chain L1, single-engine (DVE): t1''=-E[z1] copy, var,
            # Newton-1, W2 row-scale
            t1 = pt([K2, 1], dt, "t1")
            nc.vector.tensor_scalar(
                t1[:], statP[:, c0 : c0 + 1], 0.0, None, op0=AL.add
            )
            m1sq = pt([K2, 1], dt, "m1sq")
            nc.vector.tensor_mul(m1sq[:], t1[:], t1[:])
            xve1 = pt([K2, 1], dt, "xve1")
            nc.vector.scalar_tensor_tensor(
                xve1[:], statP[:, c0 + 1 : c0 + 2], 1e-5, m1sq[:],
                op0=AL.add, op1=AL.subtract,
            )
            r1 = newton1(sfx, "r1", xve1, K2, nc.vector)

            h1 = pt([K2, NJ * CH], f32r, "h1")
            if fast_path:
                s1 = r1  # gamma==1
                nc.scalar.activation(
                    h1[:, ts(0, CH)], z1sb[:, ts(0, CH)], AF.Relu, bias=t1[:]
                )
                nc.vector.tensor_scalar(
                    h1[:, ts(1, CH)], z1sb[:, ts(1, CH)], t1[:], 0.0,
                    op0=AL.add, op1=AL.max,
                )
            else:
                s1 = pt([K2, 1], dt, "s1")
                nc.vector.tensor_mul(s1[:], r1[:], W("G1R", K2))
                t1f = pt([K2, 1], dt, "t1f")
                nc.vector.scalar_tensor_tensor(
                    t1f[:], t1[:], s1[:], W("BE1R", K2),
                    op0=AL.mult, op1=AL.add,
                )
                for j in range(NJ):
                    nc.scalar.activation(
                        h1[:, ts(j, CH)], z1sb[:, ts(j, CH)], AF.Relu,
                        bias=t1f[:], scale=s1[:],
                    )

            w2p = pt([K2, K1], f32r, "w2s")
            nc.vector.tensor_scalar(
                w2p[:], W("W2BD", K2), s1[:] if fast_path else 1.0, None,
                op0=AL.mult,
            )

            for j in range(NJ):
                mm(z2P[j][:], w2p[:], h1[:, ts(j, CH)])

            z2sb = pt([K1, NJ * CH], dt, "z2sb")
            sums2 = pt([K1, 4], dt, "sums2")
            nc.scalar.activation(
                z2sb[:, ts(0, CH)], z2P[0][:], AF.Copy,
                accum_out=sums2[:, 0:1],
            )
            nc.vector.tensor_scalar(
                z2sb[:, ts(1, CH)], z2P[1][:], 0.0, 0.0, op0=AL.add,
                op1=AL.add, accum_out=sums2[:, 1:2],
            )
            s2tot = pt([K1, 1], dt, "s2tot")
            nc.gpsimd.tensor_add(s2tot[:], sums2[:, 0:1], sums2[:, 1:2])
            mm(statP[0:K1, c0 + 2 : c0 + 3], W("SR2N", K1), s2tot[:])

            sq2a = pt([K1, CH], bf16, "sq2a")
            nc.vector.scalar_tensor_tensor(
                sq2a[:], z2sb[:, ts(0, CH)], 0.0, z2sb[:, ts(0, CH)],
                op0=AL.add, op1=AL.mult, accum_out=sums2[:, 2:3],
            )
            sq2b = pt([K1, CH], bf16, "sq2b")
            nc.scalar.activation(
                sq2b[:], z2P[1][:], AF.Square, accum_out=sums2[:, 3:4]
            )
            ps2 = pt([K1, 1], dt, "ps2")
            nc.gpsimd.tensor_add(ps2[:], sums2[:, 2:3], sums2[:, 3:4])
            mm(statP[0:K1, c0 + 3 : c0 + 4], W("SR2", K1), ps2[:])

            t2 = pt([K1, 1], dt, "t2")
            nc.vector.tensor_scalar(
                t2[:], statP[0:K1, c0 + 2 : c0 + 3], 0.0, None, op0=AL.add
            )
            m2sq = pt([K1, 1], dt, "m2sq")
            nc.vector.tensor_mul(m2sq[:], t2[:], t2[:])
            xve2 = pt([K1, 1], dt, "xve2")
            nc.vector.scalar_tensor_tensor(
                xve2[:], statP[0:K1, c0 + 3 : c0 + 4], 1e-5, m2sq[:],
                op0=AL.add, op1=AL.subtract,
            )
            r2 = newton1(sfx, "r2", xve2, K1, nc.vector)
            return sfx, bix, pt, z2sb, t2, r2

        def tail(state):
            """Quantum closed form + back MLP; all inputs ready at issue."""
            sfx, bix, pt, z2sb, t2, r2 = state
            hq = pt([3, CH], f16, "hq")
            LS = pt([3, 10], f16, "LS")
            if fast_path:
                nc.scalar.activation(
                    hq[:], z2sb[0:3, 0:CH], AF.Relu, bias=t2[0:3, :]
                )
                nc.vector.tensor_scalar(
                    LS[:], W("LMAT", 3), r2[0:3, :], None, op0=AL.mult
                )
            else:
                s2 = pt([K1, 1], dt, "s2")
                nc.vector.tensor_mul(s2[:], r2[:], W("G2R", K1))
                t2f = pt([K1, 1], dt, "t2f")
                nc.vector.scalar_tensor_tensor(
                    t2f[:], t2[:], s2[:], W("BE2R", K1),
                    op0=AL.mult, op1=AL.add,
                )
                nc.scalar.activation(
                    hq[:], z2sb[0:3, 0:CH], AF.Relu, bias=t2f[0:3, :],
                    scale=s2[0:3, :],
                )
                nc.vector.tensor_scalar(
                    LS[:], WB("LMATH", 3), 1.0, None, op0=AL.mult
                )

            tp = tailP[bix % 2]
            argsP = tp[0:128, 0:TN]
            h3P = tp[0:128, TN : 2 * TN]
            h4P = tp[0:64, 2 * TN : 3 * TN]
            oP = tp[0:8, 3 * TN : 4 * TN]

            # chunk c lands at partition 32c (PE quadrant constraint)
            for c in range(TC):
                mm(argsP[32 * c : 32 * c + 10, :], LS[:],
                   hq[:, ts(c, TN)], tile_position=(0, 32 * c))
            Ct = pt([128, TN], f16, "C")
            nc.scalar.activation(
                Ct[:], argsP, AF.Sin, bias=float(np.pi / 2), scale=-1.0
            )
            mm(h3P, WB("W3KBD", 128), Ct[:])
            h3 = pt([128, TN], f16, "h3")
            nc.scalar.activation(h3[:], h3P, AF.Relu, bias=wct[:, 0:1])
            mm(h4P, WB("W4BD", 128), h3[:])
            h4 = pt([64, TN], f16, "h4")
            nc.scalar.activation(h4[:], h4P, AF.Relu, bias=wct[0:64, 1:2])
            mm(oP, WB("W5BD", 64), h4[:])
            o = pt([8, TN], dt, "o")
            nc.scalar.activation(o[:], oP, AF.Identity, bias=wct[0:8, 2:3])
            nc.sync.dma_start(out=outT[:], in_=o[:])

        SFX = ("A", "B", "C", "D")
        if loop_n > 1:
            # 2-deep software pipeline: tail(i) issues after front(i+1), so
            # tail ops (deps met) fill engine gaps while front(i+1) stalls
            # on its stats chains
            assert loop_n % 4 == 0
            with tc.For_i(0, loop_n // 4, 1):
                prev = None
                for i in range(4):
                    st = front(SFX[i], i)
                    if prev is not None:
                        tail(prev)
                    prev = st
                tail(prev)
        else:
            prev = None
            for rep in range(reps):
                st = front(SFX[rep % 4], rep % 4)
                if prev is not None:
                    tail(prev)
                prev = st
            tail(prev)

    nc.compile()
    return nc


def _wpack(inputs):
    f32 = np.float32
    a, b, t = (
        np.asarray(inputs["th1a"], f32),
        np.asarray(inputs["th1b"], f32),
        np.asarray(inputs["th2a"], f32),
    )
    ca0, sa0 = np.cos(a[0]), np.sin(a[0])
    ca1, sa1 = np.cos(a[1]), np.sin(a[1])
    cb0, sb0 = np.cos(b[0]), np.sin(b[0])
    ct0, st0 = np.cos(t[0]), np.sin(t[0])
    # xq = k0 + k1*c0 + k2*c1 + k3*s0s1 + k4*s0s2 + k5*c0s1s2
    k0 = 0.5
    k1 = -(cb0 * ca0 + ct0) / 4.0
    k2 = (sb0 * sa0 * sa1) / 4.0
    k3 = (cb0 * sa0 + st0) / 4.0
    k4 = (sb0 * ca0 * ca1) / 4.0
    k5 = (sb0 * sa0 * ca1) / 4.0
    # product-to-sum: 10 cosine terms alpha_j * cos(L_j . h)
    Lrows = np.array(
        [
            [1, 0, 0], [0, 1, 0],
            [1, -1, 0], [1, 1, 0],
            [1, 0, -1], [1, 0, 1],
            [1, -1, 1], [1, 1, -1], [1, -1, -1], [1, 1, 1],
        ],
        f32,
    )
    alpha = np.array(
        [k1, k2, k3 / 2, -k3 / 2, k4 / 2, -k4 / 2,
         k5 / 4, k5 / 4, -k5 / 4, -k5 / 4],
        f32,
    )

    wpk = np.zeros((K2, WCOLS), f32)

    def put(name, arr):
        lo, hi = _C[name]
        arr = np.asarray(arr, f32)
        if arr.ndim == 1:
            arr = arr[:, None]
        wpk[: arr.shape[0], lo:hi] = arr

    w1t = np.asarray(inputs["W1"], f32).T  # [13, 26]
    w2t = np.asarray(inputs["W2"], f32).T  # [26, 13]
    w1bd = np.zeros((K1, K2), f32)
    w2bd = np.zeros((K2, K1), f32)
    sr1 = np.tile(np.eye(26, dtype=f32), (PK, PK)) / B
    sr2 = np.tile(np.eye(NF, dtype=f32), (PK, PK)) / B
    for c in range(PK):
        w1bd[c * NF : (c + 1) * NF, c * 26 : (c + 1) * 26] = w1t
        w2bd[c * 26 : (c + 1) * 26, c * NF : (c + 1) * NF] = w2t
    put("W1BD", w1bd)
    put("W2BD", w2bd)
    put("SR1", sr1)
    put("SR2", sr2)
    put("SR1N", -sr1)
    put("SR2N", -sr2)
    put("LMAT", Lrows.T)  # [3, 10]
    put("G1R", np.tile(np.asarray(inputs["g1"], f32), PK))
    put("G2R", np.tile(np.asarray(inputs["g2"], f32), PK))
    put("BE1R", np.tile(np.asarray(inputs["beta1"], f32), PK))
    put("BE2R", np.tile(np.asarray(inputs["beta2"], f32), PK))

    # fp16 tail weights
    f16 = np.float16
    wbk = np.zeros((128, BCOLS), f16)

    def putb(name, arr):
        lo, hi = _CB[name]
        wbk[: arr.shape[0], lo:hi] = arr.astype(f16)

    W3 = np.asarray(inputs["W3"], f32)  # [32, 1]
    W4 = np.asarray(inputs["W4"], f32)  # [16, 32]
    W5 = np.asarray(inputs["W5"], f32)  # [2, 16]
    w3k = np.outer(W3[:, 0], alpha)  # [32, 10]
    w3kbd = np.zeros((128, TN), f32)
    w4bd = np.zeros((128, 16 * TC), f32)
    w5bd = np.zeros((64, 2 * TC), f32)
    for c in range(TC):
        w3kbd[32 * c : 32 * c + 10, 32 * c : 32 * (c + 1)] = w3k.T
        w4bd[32 * c : 32 * (c + 1), 16 * c : 16 * (c + 1)] = W4.T
        w5bd[16 * c : 16 * (c + 1), 2 * c : 2 * (c + 1)] = W5.T
    putb("W3KBD", w3kbd)
    putb("W4BD", w4bd)
    putb("W5BD", w5bd)
    putb("LMATH", Lrows.T)

    wck = np.zeros((128, 3), f32)
    b3p = np.asarray(inputs["b3"], f32) + W3[:, 0] * k0
    wck[:, 0] = np.tile(b3p, TC)
    wck[0:64, 1] = np.tile(np.asarray(inputs["b4"], f32), TC)
    wck[0:8, 2] = np.tile(np.asarray(inputs["b5"], f32), TC)

    return wpk, wbk, wck, w1bd


def _fast_path_ok(inputs):
    return (
        np.all(np.asarray(inputs["beta1"]) == 0)
        and np.all(np.asarray(inputs["beta2"]) == 0)
        and np.all(np.asarray(inputs["g1"]) == 1)
        and np.all(np.asarray(inputs["g2"]) == 1)
    )


def _in_maps(inputs):
    x = np.ascontiguousarray(np.asarray(inputs["x"], np.float32))
    wpk, wbk, wck, w1bd = _wpack(inputs)
    maps = []
    for c in range(NCORES):
        xr = np.roll(x, -c * SH, axis=0)
        # packed layout: xS[13*cc + f, 512*j + n] = xr[512*(PK*j + cc) + n, f]
        xs = xr.reshape(NJ, PK, CH, NF).transpose(1, 3, 0, 2).reshape(K1, NJ * CH)
        maps.append(
            {
                "xS": np.ascontiguousarray(xs), "wp": wpk, "wb": wbk,
                "wc": wck, "wr": w1bd,
            }
        )
    return maps


def run_spmd(inputs, **kw):
    from concourse import bass_utils

    nc = _build_nc(fast_path=_fast_path_ok(inputs))
    res = bass_utils.run_bass_kernel_spmd(
        nc, _in_maps(inputs), list(range(NCORES)), **kw
    )
    outs = []
    for c in range(NCORES):
        oc = res.results[c]["outT"]  # [8, 128]: rows (chunk, feat)
        outs.append(
            oc.reshape(TC, 2, TN).transpose(0, 2, 1).reshape(SH, 2)
        )
    out = np.concatenate(outs, axis=0)
    return out.astype(np.float32), res


def kernel(**inputs):
    return run_spmd(inputs)[0]


if __name__ == "__main__":
    print("built nc ok:", _build_nc() is not None)
